# revision 10
# baseline (speedup 1.0000x reference)
"""Trainium2 Bass kernel for nn_Capsule (dynamic routing capsule layer).

Math: with cij initialized to zeros, routing iteration 1 collapses to
cij = 1/32 (softmax of zeros), so the whole forward reduces to:
  T[b,j,d]   = sum_n u_hat[b,j,n,d]            (= rowsum(u[b]) @ W)
  S1         = sum(u_hat) = sum(T)
  S2         = sum(u_hat^2) = <W W^T, u^T u>   (feature Gram)
  s          = S1 * rsqrt(max(S2, 1e-12))      (global l2_normalize scalar)
  sjh2       = (s/32) * T ; sj2 = sjh2 * rsqrt(max(sum(sjh2^2), 1e-12))
  logits     = u @ As[b],  As[b][din,j] = s * sum_dd W[din,(j,dd)] sj2[b,j,dd]
  cij        = softmax_j(logits)
  G[b][j,:]  = sum_n cij[b,j,n] u[b,n,:]
  out        = squash(s * (G[b] fold W))
u_hat (256 MiB) is never materialized.  Sharding: data-parallel over
batch B (4 per core).  The cross-core reduction is 3 scalars worth of
partials (C [128x128] Gram + rowsums R), reduced on the host between
the two launches (in-kernel collectives cost ~65us here, far above the
two-launch overhead).  Layouts are host-swizzled so every DMA line is
>=2KB contiguous on both HBM and SBUF sides (the naive row-gather
yields 256B descriptor lines and ~65% of HBM bandwidth).  The logits
operand u^T is fp8 (softmax is near-uniform, |logit| <= 0.13, so fp8
quantization of u is harmless there); Gram and G operands stay bf16.
The fold-with-W + squash tail runs on the host (O(B*J*DIN*D) work) so
the second launch ends right after the G matmuls.
"""

import numpy as np

import concourse.bacc as bacc
import concourse.mybir as mybir
import concourse.tile as tile
from concourse.bass import ts
from concourse.bass_utils import run_bass_kernel_spmd

N_CORES = 8
B, N, DIN = 32, 4096, 128
J, D = 32, 16
K = J * D  # 512
B_LOC = B // N_CORES          # 4 batches per core
R_LOC = B_LOC * N             # 16384 rows per core
NCH = R_LOC // 128            # 128 chunks of 128 rows
CH_PER_B = N // 128           # 32 chunks per batch
NG1 = 8                       # phase-1 DMA groups (512 KiB each)
CHG1 = NCH // NG1             # 32 chunks per phase-1 group
NG = 8                        # phase-2 DMA groups
CHG = NCH // NG               # 16 chunks per phase-2 group
F32 = mybir.dt.float32
BF16 = mybir.dt.bfloat16
F8 = mybir.dt.float8e4
AX = mybir.AxisListType
ALU = mybir.AluOpType
ACTF = mybir.ActivationFunctionType

PROFILE = False
LAST_TIMES = {}

_CACHE = {}


def _new_bass():
    return bacc.Bacc(
        "TRN2",
        target_bir_lowering=False,
        debug=False,
        enable_asserts=True,
        num_devices=N_CORES,
    )


def _build_phase1():
    """Per core: C = sum_b u[b]^T u[b]  (feature Gram, [128,128]) and
    R[:, b] = sum_n u[b,n,:]  -> output [128, 132].

    Input u1 is host-swizzled [p, chunk, 129] bf16 where cols 0:128 are
    chunk rows and col 128 is a baked 1.0 (rides the Gram matmul to
    produce per-chunk rowsums in psum column 128)."""
    nc = _new_bass()
    u_d = nc.dram_tensor("u1", [128, NCH * 129], BF16, kind="ExternalInput")
    o_d = nc.dram_tensor("p1", [128, 132], F32, kind="ExternalOutput")

    with tile.TileContext(nc) as tc:
        with (
            tc.tile_pool(name="upool", bufs=1) as upool,
            tc.tile_pool(name="psp", bufs=1, space="PSUM") as psp,
            tc.tile_pool(name="sbp", bufs=1) as sbp,
        ):
            # HAM warm-up: ~4us of dummy matmuls while the u DMA is in
            # flight, so the PE clock is at 2.4GHz (not the cold 1.2GHz)
            # when the real matmuls start.
            wt = sbp.tile([128, 512], BF16, tag="wt", name="wt")
            nc.vector.memset(wt[:], 0.0)
            wp = psp.tile([64, 512], F32, tag="wp", name="wp")
            for _ in range(10):
                nc.tensor.matmul(wp[:], wt[:, 0:64], wt[:], start=True, stop=True)

            ugs = []
            for g in range(NG1):
                ug = upool.tile([128, CHG1 * 129], BF16, tag=f"ug{g}", name=f"ug{g}")
                ugs.append(ug)
                # spread across all three DMA-issue paths (2 HWDGE rings +
                # SWDGE) — each ring serializes its own DMAs
                eng = (nc.sync, nc.scalar, nc.gpsimd)[g % 3]
                eng.dma_start(ug[:], u_d.ap()[:, ts(g, CHG1 * 129)])

            cps = [
                psp.tile([128, 129], F32, tag=f"c{b}", name=f"c{b}")
                for b in range(B_LOC)
            ]

            for c in range(NCH):
                g, cl = divmod(c, CHG1)
                b = c // CH_PER_B
                view = ugs[g][:].rearrange("p (c e) -> p c e", e=129)[:, cl, :]
                nc.tensor.matmul(
                    cps[b][:],
                    view[:, 0:128],
                    view,
                    start=(c % CH_PER_B == 0),
                    stop=(c % CH_PER_B == CH_PER_B - 1),
                )

            outsb = sbp.tile([128, 132], F32, tag="outsb", name="outsb")
            nc.scalar.copy(outsb[:, 0:128], cps[0][:, 0:128])
            for b in range(1, B_LOC):
                nc.vector.tensor_add(
                    outsb[:, 0:128], outsb[:, 0:128], cps[b][:, 0:128]
                )
            for b in range(B_LOC):
                nc.scalar.copy(outsb[:, 128 + b : 129 + b], cps[b][:, 128:129])
            nc.sync.dma_start(o_d.ap(), outsb[:])

    nc.compile()
    return nc


def _build_phase2():
    """Per core: logits (fp8 u^T x bf16 As) -> softmax_j -> G -> out.

    out row layout: rows 32*bl+j hold G[b=core*4+bl][j, :] (din on the
    free axis).  Fold with W and squash happen on the host."""
    nc = _new_bass()
    ut_d = nc.dram_tensor("ut", [128, R_LOC], F8, kind="ExternalInput")
    u2_d = nc.dram_tensor("u2", [128, NCH * 128], BF16, kind="ExternalInput")
    a_d = nc.dram_tensor("A", [128, B_LOC * J], BF16, kind="ExternalInput")
    o_d = nc.dram_tensor("out", [128, 128], F32, kind="ExternalOutput")

    with tile.TileContext(nc) as tc:
        with (
            tc.tile_pool(name="const", bufs=1) as cstp,
            tc.tile_pool(name="utp", bufs=1) as utp,
            tc.tile_pool(name="u2p", bufs=1) as u2p,
            tc.tile_pool(name="expp", bufs=2) as expp,
            tc.tile_pool(name="cijp", bufs=3) as cijp,
            tc.tile_pool(name="zp", bufs=2) as zp,
            tc.tile_pool(name="sbt", bufs=1) as sbt,
            tc.tile_pool(name="plp", bufs=4, space="PSUM") as plp,
            tc.tile_pool(name="tlp", bufs=1, space="PSUM") as tlp,
        ):
            # HAM warm-up (see phase 1)
            wt = cstp.tile([128, 512], BF16, tag="wt", name="wt")
            nc.vector.memset(wt[:], 0.0)
            wp = tlp.tile([64, 512], F32, tag="wp", name="wp")
            for _ in range(10):
                nc.tensor.matmul(wp[:], wt[:, 0:64], wt[:], start=True, stop=True)

            # small load first so it doesn't queue behind the u loads
            a_sb = cstp.tile([128, B_LOC * J], BF16, tag="a_sb", name="a_sb")
            nc.sync.dma_start(a_sb[:], a_d.ap())

            # ut on the sync HWDGE ring, u2 on the gpsimd SWDGE path: the
            # scalar queue must stay free for EXP (a dma_start blocks the
            # issuing sequencer once its ring is full, which would push the
            # exps out past the DMA stream and stall the whole chain)
            uts, u2s = [], []
            for g in range(NG):
                ut = utp.tile([128, CHG * 128], F8, tag=f"ut{g}", name=f"ut{g}")
                uts.append(ut)
                nc.sync.dma_start(ut[:], ut_d.ap()[:, ts(g, CHG * 128)])
                u2 = u2p.tile([128, CHG * 128], BF16, tag=f"u2{g}", name=f"u2{g}")
                u2s.append(u2)
                nc.gpsimd.dma_start(u2[:], u2_d.ap()[:, ts(g, CHG * 128)])

            psg = tlp.tile([128, 128], F32, tag="psg", name="psg")  # G accum

            pls = [None] * NG
            LAG = 3  # groups of logits emitted ahead of their softmax+G chain

            def emit_logits(g):
                pls[g] = plp.tile([128, 512], F32, tag="pl", name=f"pl{g}")
                for cl in range(CHG):
                    c = g * CHG + cl
                    b = c // CH_PER_B
                    nc.tensor.matmul(
                        pls[g][:, ts(cl, J)],
                        uts[g][:, ts(cl, 128)],
                        a_sb[:, ts(b, J)],
                        start=True,
                        stop=True,
                    )

            def emit_chain(g):
                # softmax over j (free axis) + G matmuls for group g
                eg = expp.tile([128, 512], BF16, tag="eg", name=f"eg{g}")
                nc.scalar.activation(eg[:], pls[g][:], ACTF.Exp)
                zg = zp.tile([128, CHG], BF16, tag="zg", name=f"zg{g}")
                zr = zp.tile([128, CHG], BF16, tag="zr", name=f"zr{g}")
                cg = cijp.tile([128, 512], BF16, tag="cg", name=f"cg{g}")
                with nc.allow_low_precision(
                    reason="softmax denominator: 32-way sum of O(1) exps; "
                    "bf16 keeps the full DVE 2x 16-bit rate"
                ):
                    nc.vector.reduce_sum(
                        zg[:], eg[:].rearrange("p (c j) -> p c j", j=J), axis=AX.X
                    )
                    nc.vector.reciprocal(zr[:], zg[:])
                    nc.vector.tensor_tensor(
                        cg[:].rearrange("p (c j) -> p c j", j=J),
                        eg[:].rearrange("p (c j) -> p c j", j=J),
                        zr[:].unsqueeze(2).broadcast_to([128, CHG, J]),
                        op=ALU.mult,
                    )
                for cc in range(CHG):
                    c2 = g * CHG + cc
                    b2 = c2 // CH_PER_B
                    nc.tensor.matmul(
                        psg[ts(b2, J), :],
                        cg[:, ts(cc, J)],
                        u2s[g][:, ts(cc, 128)],
                        start=(c2 % CH_PER_B == 0),
                        stop=(c2 % CH_PER_B == CH_PER_B - 1),
                        tile_position=(0, 32 * b2),
                    )

            for g in range(NG):
                emit_logits(g)
                if g >= LAG:
                    emit_chain(g - LAG)
            for g in range(NG - LAG, NG):
                emit_chain(g)

            gout = sbt.tile([128, 128], F32, tag="gout", name="gout")
            nc.scalar.copy(gout[:], psg[:])
            nc.sync.dma_start(o_d.ap(), gout[:])

    nc.compile()
    return nc


def _get(name):
    if name not in _CACHE:
        if name == "p1":
            _CACHE[name] = _build_phase1()
        else:
            _CACHE[name] = _build_phase2()
    return _CACHE[name]


def kernel(u, W):
    import ml_dtypes

    bf16 = ml_dtypes.bfloat16
    f8 = ml_dtypes.float8_e4m3
    u = np.ascontiguousarray(u, dtype=np.float32)
    W = np.ascontiguousarray(W, dtype=np.float32)
    W0 = np.ascontiguousarray(W[0])  # [128, 512]

    u1s, u2s, ut8s = [], [], []
    for i in range(N_CORES):
        shf = u[i * B_LOC : (i + 1) * B_LOC].reshape(R_LOC, DIN)
        shc = shf.astype(bf16).reshape(NCH, 128, DIN)  # [c, p, d]
        sw = shc.transpose(1, 0, 2)  # [p, c, d]
        u1 = np.empty((128, NCH, 129), dtype=bf16)
        u1[:, :, 0:128] = sw
        u1[:, :, 128] = bf16(1.0)
        u1s.append(np.ascontiguousarray(u1.reshape(128, NCH * 129)))
        u2s.append(np.ascontiguousarray(sw.reshape(128, NCH * 128)))
        ut8s.append(np.ascontiguousarray(shf.T).astype(f8))

    # ---- phase 1: per-core Gram + rowsums ----
    nc1 = _get("p1")
    r1 = run_bass_kernel_spmd(
        nc1,
        [{"u1": u1s[i]} for i in range(N_CORES)],
        core_ids=list(range(N_CORES)),
        trace=PROFILE,
    )
    if PROFILE:
        LAST_TIMES["phase1_ns"] = r1.exec_time_ns

    # ---- host: global scalar reduction (the "all-reduce" of 3 scalars) ----
    C = np.zeros((128, 128), dtype=np.float64)
    Rall = np.empty((128, B), dtype=np.float64)
    for i in range(N_CORES):
        p = r1.results[i]["p1"].astype(np.float64)
        C += p[:, :128]
        Rall[:, i * B_LOC : (i + 1) * B_LOC] = p[:, 128:132]
    W0d = W0.astype(np.float64)
    M = W0d @ W0d.T
    S2 = float(np.vdot(M, C))
    T = Rall.T @ W0d  # [B, 512]
    S1 = float(T.sum())
    s = S1 / np.sqrt(max(S2, 1e-12))
    sjh2 = (s / J) * T
    n2 = float((sjh2 * sjh2).sum())
    sj2 = (sjh2 / np.sqrt(max(n2, 1e-12))).reshape(B, J, D)
    # As[b][din, j] = s * sum_dd W0[din, (j,dd)] * sj2[b, j, dd]
    A = np.einsum("dje,bje->bdj", W0d.reshape(DIN, J, D), sj2)
    As = (s * A).astype(bf16)  # [B, 128, 32]

    # ---- phase 2: logits/softmax/G ----
    nc2 = _get("p2")
    in2 = [
        {
            "ut": ut8s[i],
            "u2": u2s[i],
            "A": np.ascontiguousarray(
                As[i * B_LOC : (i + 1) * B_LOC].transpose(1, 0, 2).reshape(
                    DIN, B_LOC * J
                )
            ),
        }
        for i in range(N_CORES)
    ]
    r2 = run_bass_kernel_spmd(
        nc2, in2, core_ids=list(range(N_CORES)), trace=PROFILE
    )
    if PROFILE:
        LAST_TIMES["phase2_ns"] = r2.exec_time_ns

    # ---- host: fold G with W + squash ----
    W0r = W0d.reshape(DIN, J, D)
    out = np.empty((B, J, D), dtype=np.float32)
    for i in range(N_CORES):
        Gi = r2.results[i]["out"].astype(np.float64)  # [128, 128]
        for bl in range(B_LOC):
            Gb = Gi[32 * bl : 32 * bl + 32, :]  # [j, din]
            sjh3 = s * np.einsum("jd,djk->jk", Gb, W0r)
            s2 = (sjh3 * sjh3).sum(axis=-1, keepdims=True) + 1e-7
            out[i * B_LOC + bl] = (np.sqrt(s2) / (1.0 + s2)) * sjh3
    return out


# revision 13
# speedup vs baseline: 1.0371x; 1.0371x over previous
"""Trainium2 Bass kernel for nn_Capsule (dynamic routing capsule layer).

Math: with cij initialized to zeros, routing iteration 1 collapses to
cij = 1/32 (softmax of zeros), so the whole forward reduces to:
  T[b,j,d]   = sum_n u_hat[b,j,n,d]            (= rowsum(u[b]) @ W)
  S1         = sum(u_hat) = sum(T)
  S2         = sum(u_hat^2) = <W W^T, u^T u>   (feature Gram)
  s          = S1 * rsqrt(max(S2, 1e-12))      (global l2_normalize scalar)
  sjh2       = (s/32) * T ; sj2 = sjh2 * rsqrt(max(sum(sjh2^2), 1e-12))
  logits     = u @ As[b],  As[b][din,j] = s * sum_dd W[din,(j,dd)] sj2[b,j,dd]
  cij        = softmax_j(logits)
  G[b][j,:]  = sum_n cij[b,j,n] u[b,n,:]
  out        = squash(s * (G[b] fold W))
u_hat (256 MiB) is never materialized.  Sharding: data-parallel over
batch B (4 per core).  The cross-core reduction is 3 scalars worth of
partials (C [128x128] Gram + rowsums R), reduced on the host between
the two launches (in-kernel collectives cost ~65us here, far above the
two-launch overhead).  Layouts are host-swizzled so every DMA line is
>=2KB contiguous on both HBM and SBUF sides (the naive row-gather
yields 256B descriptor lines and ~65% of HBM bandwidth).  The logits
operand u^T is fp8 (softmax is near-uniform, |logit| <= 0.13, so fp8
quantization of u is harmless there); Gram and G operands stay bf16.
The fold-with-W + squash tail runs on the host (O(B*J*DIN*D) work) so
the second launch ends right after the G matmuls.
"""

import numpy as np

import concourse.bacc as bacc
import concourse.mybir as mybir
import concourse.tile as tile
from concourse.bass import ts
from concourse.bass_utils import run_bass_kernel_spmd

N_CORES = 8
B, N, DIN = 32, 4096, 128
J, D = 32, 16
K = J * D  # 512
B_LOC = B // N_CORES          # 4 batches per core
R_LOC = B_LOC * N             # 16384 rows per core
NCH = R_LOC // 128            # 128 chunks of 128 rows
CH_PER_B = N // 128           # 32 chunks per batch
NG1 = 8                       # phase-1 DMA groups (512 KiB each)
CHG1 = NCH // NG1             # 32 chunks per phase-1 group
NG = 8                        # phase-2 DMA groups
CHG = NCH // NG               # 16 chunks per phase-2 group
F32 = mybir.dt.float32
BF16 = mybir.dt.bfloat16
F8 = mybir.dt.float8e4
AX = mybir.AxisListType
ALU = mybir.AluOpType
ACTF = mybir.ActivationFunctionType

PROFILE = False
LAST_TIMES = {}

_CACHE = {}


def _new_bass():
    return bacc.Bacc(
        "TRN2",
        target_bir_lowering=False,
        debug=False,
        enable_asserts=True,
        num_devices=N_CORES,
    )


def _build_phase1():
    """Per core: C = sum_b u[b]^T u[b]  (feature Gram, [128,128]) and
    R[:, b] = sum_n u[b,n,:]  -> output [128, 132].

    Input u1 is host-swizzled [p, chunk, 129] bf16 where cols 0:128 are
    chunk rows and col 128 is a baked 1.0 (rides the Gram matmul to
    produce per-chunk rowsums in psum column 128)."""
    nc = _new_bass()
    u_d = nc.dram_tensor("u1", [128, NCH * 129], BF16, kind="ExternalInput")
    o_d = nc.dram_tensor("p1", [128, 132], F32, kind="ExternalOutput")

    with tile.TileContext(nc) as tc:
        with (
            tc.tile_pool(name="upool", bufs=1) as upool,
            tc.tile_pool(name="psp", bufs=1, space="PSUM") as psp,
            tc.tile_pool(name="sbp", bufs=1) as sbp,
        ):
            # HAM warm-up: ~4us of dummy matmuls while the u DMA is in
            # flight, so the PE clock is at 2.4GHz (not the cold 1.2GHz)
            # when the real matmuls start.
            wt = sbp.tile([128, 512], BF16, tag="wt", name="wt")
            nc.vector.memset(wt[:], 0.0)
            wp = psp.tile([64, 512], F32, tag="wp", name="wp")
            for _ in range(10):
                nc.tensor.matmul(wp[:], wt[:, 0:64], wt[:], start=True, stop=True)

            ugs = []
            for g in range(NG1):
                ug = upool.tile([128, CHG1 * 129], BF16, tag=f"ug{g}", name=f"ug{g}")
                ugs.append(ug)
                # alternate the two HWDGE rings; SWDGE (gpsimd) transfers
                # serialize at ~3us each and arrive far too late
                eng = nc.sync if g % 2 == 0 else nc.scalar
                eng.dma_start(ug[:], u_d.ap()[:, ts(g, CHG1 * 129)])

            cps = [
                psp.tile([128, 129], F32, tag=f"c{b}", name=f"c{b}")
                for b in range(B_LOC)
            ]

            for c in range(NCH):
                g, cl = divmod(c, CHG1)
                b = c // CH_PER_B
                view = ugs[g][:].rearrange("p (c e) -> p c e", e=129)[:, cl, :]
                nc.tensor.matmul(
                    cps[b][:],
                    view[:, 0:128],
                    view,
                    start=(c % CH_PER_B == 0),
                    stop=(c % CH_PER_B == CH_PER_B - 1),
                )

            outsb = sbp.tile([128, 132], F32, tag="outsb", name="outsb")
            nc.scalar.copy(outsb[:, 0:128], cps[0][:, 0:128])
            for b in range(1, B_LOC):
                nc.vector.tensor_add(
                    outsb[:, 0:128], outsb[:, 0:128], cps[b][:, 0:128]
                )
            for b in range(B_LOC):
                nc.scalar.copy(outsb[:, 128 + b : 129 + b], cps[b][:, 128:129])
            nc.sync.dma_start(o_d.ap(), outsb[:])

    nc.compile()
    return nc


def _build_phase2():
    """Per core: logits (fp8 u^T x bf16 As) -> softmax_j -> G -> out.

    out row layout: rows 32*bl+j hold G[b=core*4+bl][j, :] (din on the
    free axis).  Fold with W and squash happen on the host."""
    nc = _new_bass()
    ut_d = nc.dram_tensor("ut", [128, R_LOC], F8, kind="ExternalInput")
    u2_d = nc.dram_tensor("u2", [128, NCH * 128], BF16, kind="ExternalInput")
    a_d = nc.dram_tensor("A", [128, B_LOC * J], BF16, kind="ExternalInput")
    o_d = nc.dram_tensor("out", [128, 128], F32, kind="ExternalOutput")

    with tile.TileContext(nc) as tc:
        with (
            tc.tile_pool(name="const", bufs=1) as cstp,
            tc.tile_pool(name="utp", bufs=1) as utp,
            tc.tile_pool(name="u2p", bufs=1) as u2p,
            tc.tile_pool(name="expp", bufs=2) as expp,
            tc.tile_pool(name="cijp", bufs=3) as cijp,
            tc.tile_pool(name="zp", bufs=2) as zp,
            tc.tile_pool(name="sbt", bufs=1) as sbt,
            tc.tile_pool(name="plp", bufs=4, space="PSUM") as plp,
            tc.tile_pool(name="tlp", bufs=1, space="PSUM") as tlp,
        ):
            # HAM warm-up (see phase 1)
            wt = cstp.tile([128, 512], BF16, tag="wt", name="wt")
            nc.vector.memset(wt[:], 0.0)
            wp = tlp.tile([64, 512], F32, tag="wp", name="wp")
            for _ in range(10):
                nc.tensor.matmul(wp[:], wt[:, 0:64], wt[:], start=True, stop=True)

            # small load first so it doesn't queue behind the u loads
            a_sb = cstp.tile([128, B_LOC * J], BF16, tag="a_sb", name="a_sb")
            nc.sync.dma_start(a_sb[:], a_d.ap())

            # ut on the sync HWDGE ring; u2 on the scalar ring, but only the
            # first 3 issued up front — the rest are interleaved between the
            # EXPs below so a full ring never blocks an EXP behind it in the
            # scalar queue (SWDGE/gpsimd is not an option: its transfers
            # serialize at ~3us each and arrive far too late)
            uts, u2s = [], []
            for g in range(NG):
                ut = utp.tile([128, CHG * 128], F8, tag=f"ut{g}", name=f"ut{g}")
                uts.append(ut)
                nc.sync.dma_start(ut[:], ut_d.ap()[:, ts(g, CHG * 128)])
                u2 = u2p.tile([128, CHG * 128], BF16, tag=f"u2{g}", name=f"u2{g}")
                u2s.append(u2)
            U2_HEAD = 3
            for g in range(U2_HEAD):
                nc.scalar.dma_start(u2s[g][:], u2_d.ap()[:, ts(g, CHG * 128)])

            psg = tlp.tile([128, 128], F32, tag="psg", name="psg")  # G accum

            pls = [None] * NG
            LAG = 3  # groups of logits emitted ahead of their softmax+G chain

            def emit_logits(g):
                pls[g] = plp.tile([128, 512], F32, tag="pl", name=f"pl{g}")
                for cl in range(CHG):
                    c = g * CHG + cl
                    b = c // CH_PER_B
                    nc.tensor.matmul(
                        pls[g][:, ts(cl, J)],
                        uts[g][:, ts(cl, 128)],
                        a_sb[:, ts(b, J)],
                        start=True,
                        stop=True,
                    )

            def emit_chain(g):
                # softmax over j (free axis) + G matmuls for group g
                eg = expp.tile([128, 512], BF16, tag="eg", name=f"eg{g}")
                nc.scalar.activation(eg[:], pls[g][:], ACTF.Exp)
                zg = zp.tile([128, CHG], BF16, tag="zg", name=f"zg{g}")
                zr = zp.tile([128, CHG], BF16, tag="zr", name=f"zr{g}")
                cg = cijp.tile([128, 512], BF16, tag="cg", name=f"cg{g}")
                with nc.allow_low_precision(
                    reason="softmax denominator: 32-way sum of O(1) exps; "
                    "bf16 keeps the full DVE 2x 16-bit rate"
                ):
                    nc.vector.reduce_sum(
                        zg[:], eg[:].rearrange("p (c j) -> p c j", j=J), axis=AX.X
                    )
                    nc.vector.reciprocal(zr[:], zg[:])
                    nc.vector.tensor_tensor(
                        cg[:].rearrange("p (c j) -> p c j", j=J),
                        eg[:].rearrange("p (c j) -> p c j", j=J),
                        zr[:].unsqueeze(2).broadcast_to([128, CHG, J]),
                        op=ALU.mult,
                    )
                for cc in range(CHG):
                    c2 = g * CHG + cc
                    b2 = c2 // CH_PER_B
                    nc.tensor.matmul(
                        psg[ts(b2, J), :],
                        cg[:, ts(cc, J)],
                        u2s[g][:, ts(cc, 128)],
                        start=(c2 % CH_PER_B == 0),
                        stop=(c2 % CH_PER_B == CH_PER_B - 1),
                        tile_position=(0, 32 * b2),
                    )

            def emit_u2_dma(g):
                if U2_HEAD <= g < NG:
                    nc.scalar.dma_start(u2s[g][:], u2_d.ap()[:, ts(g, CHG * 128)])

            for g in range(NG):
                emit_logits(g)
                if g >= LAG:
                    emit_chain(g - LAG)
                    emit_u2_dma(g - LAG + U2_HEAD)
            for g in range(NG - LAG, NG):
                emit_chain(g)
                emit_u2_dma(g + U2_HEAD)

            gout = sbt.tile([128, 128], F32, tag="gout", name="gout")
            nc.scalar.copy(gout[:], psg[:])
            nc.sync.dma_start(o_d.ap(), gout[:])

    nc.compile()
    return nc


def _get(name):
    if name not in _CACHE:
        if name == "p1":
            _CACHE[name] = _build_phase1()
        else:
            _CACHE[name] = _build_phase2()
    return _CACHE[name]


def kernel(u, W):
    import ml_dtypes

    bf16 = ml_dtypes.bfloat16
    f8 = ml_dtypes.float8_e4m3
    u = np.ascontiguousarray(u, dtype=np.float32)
    W = np.ascontiguousarray(W, dtype=np.float32)
    W0 = np.ascontiguousarray(W[0])  # [128, 512]

    u1s, u2s, ut8s = [], [], []
    for i in range(N_CORES):
        shf = u[i * B_LOC : (i + 1) * B_LOC].reshape(R_LOC, DIN)
        shc = shf.astype(bf16).reshape(NCH, 128, DIN)  # [c, p, d]
        sw = shc.transpose(1, 0, 2)  # [p, c, d]
        u1 = np.empty((128, NCH, 129), dtype=bf16)
        u1[:, :, 0:128] = sw
        u1[:, :, 128] = bf16(1.0)
        u1s.append(np.ascontiguousarray(u1.reshape(128, NCH * 129)))
        u2s.append(np.ascontiguousarray(sw.reshape(128, NCH * 128)))
        ut8s.append(np.ascontiguousarray(shf.T).astype(f8))

    # ---- phase 1: per-core Gram + rowsums ----
    nc1 = _get("p1")
    r1 = run_bass_kernel_spmd(
        nc1,
        [{"u1": u1s[i]} for i in range(N_CORES)],
        core_ids=list(range(N_CORES)),
        trace=PROFILE,
    )
    if PROFILE:
        LAST_TIMES["phase1_ns"] = r1.exec_time_ns

    # ---- host: global scalar reduction (the "all-reduce" of 3 scalars) ----
    C = np.zeros((128, 128), dtype=np.float64)
    Rall = np.empty((128, B), dtype=np.float64)
    for i in range(N_CORES):
        p = r1.results[i]["p1"].astype(np.float64)
        C += p[:, :128]
        Rall[:, i * B_LOC : (i + 1) * B_LOC] = p[:, 128:132]
    W0d = W0.astype(np.float64)
    M = W0d @ W0d.T
    S2 = float(np.vdot(M, C))
    T = Rall.T @ W0d  # [B, 512]
    S1 = float(T.sum())
    s = S1 / np.sqrt(max(S2, 1e-12))
    sjh2 = (s / J) * T
    n2 = float((sjh2 * sjh2).sum())
    sj2 = (sjh2 / np.sqrt(max(n2, 1e-12))).reshape(B, J, D)
    # As[b][din, j] = s * sum_dd W0[din, (j,dd)] * sj2[b, j, dd]
    A = np.einsum("dje,bje->bdj", W0d.reshape(DIN, J, D), sj2)
    As = (s * A).astype(bf16)  # [B, 128, 32]

    # ---- phase 2: logits/softmax/G ----
    nc2 = _get("p2")
    in2 = [
        {
            "ut": ut8s[i],
            "u2": u2s[i],
            "A": np.ascontiguousarray(
                As[i * B_LOC : (i + 1) * B_LOC].transpose(1, 0, 2).reshape(
                    DIN, B_LOC * J
                )
            ),
        }
        for i in range(N_CORES)
    ]
    r2 = run_bass_kernel_spmd(
        nc2, in2, core_ids=list(range(N_CORES)), trace=PROFILE
    )
    if PROFILE:
        LAST_TIMES["phase2_ns"] = r2.exec_time_ns

    # ---- host: fold G with W + squash ----
    W0r = W0d.reshape(DIN, J, D)
    out = np.empty((B, J, D), dtype=np.float32)
    for i in range(N_CORES):
        Gi = r2.results[i]["out"].astype(np.float64)  # [128, 128]
        for bl in range(B_LOC):
            Gb = Gi[32 * bl : 32 * bl + 32, :]  # [j, din]
            sjh3 = s * np.einsum("jd,djk->jk", Gb, W0r)
            s2 = (sjh3 * sjh3).sum(axis=-1, keepdims=True) + 1e-7
            out[i * B_LOC + bl] = (np.sqrt(s2) / (1.0 + s2)) * sjh3
    return out


# revision 15
# speedup vs baseline: 1.1141x; 1.0742x over previous
"""Trainium2 Bass kernel for nn_Capsule (dynamic routing capsule layer).

Math: with cij initialized to zeros, routing iteration 1 collapses to
cij = 1/32 (softmax of zeros), so the whole forward reduces to:
  T[b,j,d]   = sum_n u_hat[b,j,n,d]            (= rowsum(u[b]) @ W)
  S1         = sum(u_hat) = sum(T)
  S2         = sum(u_hat^2) = <W W^T, u^T u>   (feature Gram)
  s          = S1 * rsqrt(max(S2, 1e-12))      (global l2_normalize scalar)
  sjh2       = (s/32) * T ; sj2 = sjh2 * rsqrt(max(sum(sjh2^2), 1e-12))
  logits     = u @ As[b],  As[b][din,j] = s * sum_dd W[din,(j,dd)] sj2[b,j,dd]
  cij        = softmax_j(logits)
  G[b][j,:]  = sum_n cij[b,j,n] u[b,n,:]
  out        = squash(s * (G[b] fold W))
u_hat (256 MiB) is never materialized.  Sharding: data-parallel over
batch B (4 per core).  The cross-core reduction is 3 scalars worth of
partials (C [128x128] Gram + rowsums R), reduced on the host between
the two launches (in-kernel collectives cost ~65us here, far above the
two-launch overhead).  Layouts are host-swizzled so every DMA line is
>=2KB contiguous on both HBM and SBUF sides (the naive row-gather
yields 256B descriptor lines and ~65% of HBM bandwidth).  The logits
operand u^T is fp8 (softmax is near-uniform, |logit| <= 0.13, so fp8
quantization of u is harmless there); Gram and G operands stay bf16.
The fold-with-W + squash tail runs on the host (O(B*J*DIN*D) work) so
the second launch ends right after the G matmuls.
"""

import numpy as np

import concourse.bacc as bacc
import concourse.mybir as mybir
import concourse.tile as tile
from concourse.bass import ts
from concourse.bass_utils import run_bass_kernel_spmd

N_CORES = 8
B, N, DIN = 32, 4096, 128
J, D = 32, 16
K = J * D  # 512
B_LOC = B // N_CORES          # 4 batches per core
R_LOC = B_LOC * N             # 16384 rows per core
NCH = R_LOC // 128            # 128 chunks of 128 rows
CH_PER_B = N // 128           # 32 chunks per batch
NG1 = 8                       # phase-1 DMA groups (512 KiB each)
CHG1 = NCH // NG1             # 32 chunks per phase-1 group
NG = 8                        # phase-2 DMA groups
CHG = NCH // NG               # 16 chunks per phase-2 group
F32 = mybir.dt.float32
BF16 = mybir.dt.bfloat16
F8 = mybir.dt.float8e4
AX = mybir.AxisListType
ALU = mybir.AluOpType
ACTF = mybir.ActivationFunctionType

PROFILE = False
LAST_TIMES = {}

_CACHE = {}


def _new_bass():
    return bacc.Bacc(
        "TRN2",
        target_bir_lowering=False,
        debug=False,
        enable_asserts=True,
        num_devices=N_CORES,
    )


def _build_phase1():
    """Per core: C = sum_b u[b]^T u[b]  (feature Gram, [128,128]) and
    R[:, b] = sum_n u[b,n,:]  -> output [128, 132].

    Input u1 is host-swizzled [p, chunk, 129] bf16 where cols 0:128 are
    chunk rows and col 128 is a baked 1.0 (rides the Gram matmul to
    produce per-chunk rowsums in psum column 128)."""
    nc = _new_bass()
    u_d = nc.dram_tensor("u1", [128, NCH * 129], BF16, kind="ExternalInput")
    o_d = nc.dram_tensor("p1", [128, 132], F32, kind="ExternalOutput")

    with tile.TileContext(nc) as tc:
        with (
            tc.tile_pool(name="upool", bufs=1) as upool,
            tc.tile_pool(name="psp", bufs=1, space="PSUM") as psp,
            tc.tile_pool(name="sbp", bufs=1) as sbp,
        ):
            # HAM warm-up: ~4us of dummy matmuls while the u DMA is in
            # flight, so the PE clock is at 2.4GHz (not the cold 1.2GHz)
            # when the real matmuls start.
            wt = sbp.tile([128, 512], BF16, tag="wt", name="wt")
            nc.vector.memset(wt[:], 0.0)
            wp = psp.tile([64, 512], F32, tag="wp", name="wp")
            for _ in range(6):
                nc.tensor.matmul(wp[:], wt[:, 0:64], wt[:], start=True, stop=True)

            ugs = []
            for g in range(NG1):
                ug = upool.tile([128, CHG1 * 129], BF16, tag=f"ug{g}", name=f"ug{g}")
                ugs.append(ug)
                # alternate the two HWDGE rings; SWDGE (gpsimd) transfers
                # serialize at ~3us each and arrive far too late
                eng = nc.sync if g % 2 == 0 else nc.scalar
                eng.dma_start(ug[:], u_d.ap()[:, ts(g, CHG1 * 129)])

            cps = [
                psp.tile([128, 129], F32, tag=f"c{b}", name=f"c{b}")
                for b in range(B_LOC)
            ]

            for c in range(NCH):
                g, cl = divmod(c, CHG1)
                b = c // CH_PER_B
                view = ugs[g][:].rearrange("p (c e) -> p c e", e=129)[:, cl, :]
                nc.tensor.matmul(
                    cps[b][:],
                    view[:, 0:128],
                    view,
                    start=(c % CH_PER_B == 0),
                    stop=(c % CH_PER_B == CH_PER_B - 1),
                )

            outsb = sbp.tile([128, 132], F32, tag="outsb", name="outsb")
            nc.scalar.copy(outsb[:, 0:128], cps[0][:, 0:128])
            for b in range(1, B_LOC):
                nc.vector.tensor_add(
                    outsb[:, 0:128], outsb[:, 0:128], cps[b][:, 0:128]
                )
            for b in range(B_LOC):
                nc.scalar.copy(outsb[:, 128 + b : 129 + b], cps[b][:, 128:129])
            nc.sync.dma_start(o_d.ap(), outsb[:])

    nc.compile()
    return nc


def _build_phase2():
    """Per core: logits (fp8 u^T x bf16 As) -> softmax_j -> G -> out.

    out row layout: rows 32*bl+j hold G[b=core*4+bl][j, :] (din on the
    free axis).  Fold with W and squash happen on the host."""
    nc = _new_bass()
    ut_d = nc.dram_tensor("ut", [128, R_LOC], F8, kind="ExternalInput")
    u2_d = nc.dram_tensor("u2", [128, NCH * 128], BF16, kind="ExternalInput")
    a_d = nc.dram_tensor("A", [128, B_LOC * J], BF16, kind="ExternalInput")
    o_d = nc.dram_tensor("out", [128, 128], F32, kind="ExternalOutput")

    with tile.TileContext(nc) as tc:
        with (
            tc.tile_pool(name="const", bufs=1) as cstp,
            tc.tile_pool(name="utp", bufs=1) as utp,
            tc.tile_pool(name="u2p", bufs=1) as u2p,
            tc.tile_pool(name="expp", bufs=2) as expp,
            tc.tile_pool(name="cijp", bufs=3) as cijp,
            tc.tile_pool(name="zp", bufs=2) as zp,
            tc.tile_pool(name="sbt", bufs=1) as sbt,
            tc.tile_pool(name="plp", bufs=4, space="PSUM") as plp,
            tc.tile_pool(name="tlp", bufs=1, space="PSUM") as tlp,
        ):
            # HAM warm-up (see phase 1)
            wt = cstp.tile([128, 512], BF16, tag="wt", name="wt")
            nc.vector.memset(wt[:], 0.0)
            wp = tlp.tile([64, 512], F32, tag="wp", name="wp")
            for _ in range(6):
                nc.tensor.matmul(wp[:], wt[:, 0:64], wt[:], start=True, stop=True)

            # small load first so it doesn't queue behind the u loads
            a_sb = cstp.tile([128, B_LOC * J], BF16, tag="a_sb", name="a_sb")
            nc.sync.dma_start(a_sb[:], a_d.ap())

            # graduated group sizes: the serial exp->softmax->G chain for the
            # LAST group runs after the final DMA byte, so the tail groups
            # shrink (16...16,8,4,4 chunks) to cut that exposed latency
            GROUPS = [(0, 16), (16, 16), (32, 16), (48, 16), (64, 16),
                      (80, 16), (96, 16), (112, 8), (120, 4), (124, 4)]
            NGV = len(GROUPS)

            # ut on the sync HWDGE ring; u2 on the scalar ring, but only the
            # first 3 issued up front — the rest are interleaved between the
            # EXPs below so a full ring never blocks an EXP behind it in the
            # scalar queue (SWDGE/gpsimd is not an option: its transfers
            # serialize at ~3us each and arrive far too late)
            uts, u2s = [], []
            for g, (c0, ng) in enumerate(GROUPS):
                ut = utp.tile([128, ng * 128], F8, tag=f"ut{g}", name=f"ut{g}")
                uts.append(ut)
                nc.sync.dma_start(ut[:], ut_d.ap()[:, c0 * 128 : (c0 + ng) * 128])
                u2 = u2p.tile([128, ng * 128], BF16, tag=f"u2{g}", name=f"u2{g}")
                u2s.append(u2)
            U2_HEAD = 3
            for g in range(U2_HEAD):
                c0, ng = GROUPS[g]
                nc.scalar.dma_start(u2s[g][:], u2_d.ap()[:, c0 * 128 : (c0 + ng) * 128])

            psg = tlp.tile([128, 128], F32, tag="psg", name="psg")  # G accum

            pls = [None] * NGV
            LAG = 3  # groups of logits emitted ahead of their softmax+G chain

            def emit_logits(g):
                c0, ng = GROUPS[g]
                pls[g] = plp.tile([128, 512], F32, tag="pl", name=f"pl{g}")
                for cl in range(ng):
                    b = (c0 + cl) // CH_PER_B
                    nc.tensor.matmul(
                        pls[g][:, ts(cl, J)],
                        uts[g][:, ts(cl, 128)],
                        a_sb[:, ts(b, J)],
                        start=True,
                        stop=True,
                    )

            def emit_chain(g):
                # softmax over j (free axis) + G matmuls for group g
                c0, ng = GROUPS[g]
                eg = expp.tile([128, 512], BF16, tag="eg", name=f"eg{g}")
                nc.scalar.activation(eg[:, : ng * J], pls[g][:, : ng * J], ACTF.Exp)
                zg = zp.tile([128, 16], BF16, tag="zg", name=f"zg{g}")
                zr = zp.tile([128, 16], BF16, tag="zr", name=f"zr{g}")
                cg = cijp.tile([128, 512], BF16, tag="cg", name=f"cg{g}")
                with nc.allow_low_precision(
                    reason="softmax denominator: 32-way sum of O(1) exps; "
                    "bf16 keeps the full DVE 2x 16-bit rate"
                ):
                    nc.vector.reduce_sum(
                        zg[:, :ng],
                        eg[:, : ng * J].rearrange("p (c j) -> p c j", j=J),
                        axis=AX.X,
                    )
                    nc.vector.reciprocal(zr[:, :ng], zg[:, :ng])
                    # the big elementwise multiply runs on the otherwise-idle
                    # gpsimd engine so the vector engine stays off the
                    # critical path
                    nc.gpsimd.tensor_tensor(
                        cg[:, : ng * J].rearrange("p (c j) -> p c j", j=J),
                        eg[:, : ng * J].rearrange("p (c j) -> p c j", j=J),
                        zr[:, :ng].unsqueeze(2).broadcast_to([128, ng, J]),
                        op=ALU.mult,
                    )
                for cc in range(ng):
                    c2 = c0 + cc
                    b2 = c2 // CH_PER_B
                    nc.tensor.matmul(
                        psg[ts(b2, J), :],
                        cg[:, ts(cc, J)],
                        u2s[g][:, ts(cc, 128)],
                        start=(c2 % CH_PER_B == 0),
                        stop=(c2 % CH_PER_B == CH_PER_B - 1),
                        tile_position=(0, 32 * b2),
                    )

            def emit_u2_dma(g):
                if U2_HEAD <= g < NGV:
                    c0, ng = GROUPS[g]
                    nc.scalar.dma_start(
                        u2s[g][:], u2_d.ap()[:, c0 * 128 : (c0 + ng) * 128]
                    )

            for g in range(NGV):
                emit_logits(g)
                if g >= LAG:
                    emit_chain(g - LAG)
                    emit_u2_dma(g - LAG + U2_HEAD)
            for g in range(NGV - LAG, NGV):
                emit_chain(g)
                emit_u2_dma(g + U2_HEAD)

            gout = sbt.tile([128, 128], F32, tag="gout", name="gout")
            nc.scalar.copy(gout[:], psg[:])
            nc.sync.dma_start(o_d.ap(), gout[:])

    nc.compile()
    return nc


def _get(name):
    if name not in _CACHE:
        if name == "p1":
            _CACHE[name] = _build_phase1()
        else:
            _CACHE[name] = _build_phase2()
    return _CACHE[name]


def kernel(u, W):
    import ml_dtypes

    bf16 = ml_dtypes.bfloat16
    f8 = ml_dtypes.float8_e4m3
    u = np.ascontiguousarray(u, dtype=np.float32)
    W = np.ascontiguousarray(W, dtype=np.float32)
    W0 = np.ascontiguousarray(W[0])  # [128, 512]

    u1s, u2s, ut8s = [], [], []
    for i in range(N_CORES):
        shf = u[i * B_LOC : (i + 1) * B_LOC].reshape(R_LOC, DIN)
        shc = shf.astype(bf16).reshape(NCH, 128, DIN)  # [c, p, d]
        sw = shc.transpose(1, 0, 2)  # [p, c, d]
        u1 = np.empty((128, NCH, 129), dtype=bf16)
        u1[:, :, 0:128] = sw
        u1[:, :, 128] = bf16(1.0)
        u1s.append(np.ascontiguousarray(u1.reshape(128, NCH * 129)))
        u2s.append(np.ascontiguousarray(sw.reshape(128, NCH * 128)))
        ut8s.append(np.ascontiguousarray(shf.T).astype(f8))

    # ---- phase 1: per-core Gram + rowsums ----
    nc1 = _get("p1")
    r1 = run_bass_kernel_spmd(
        nc1,
        [{"u1": u1s[i]} for i in range(N_CORES)],
        core_ids=list(range(N_CORES)),
        trace=PROFILE,
    )
    if PROFILE:
        LAST_TIMES["phase1_ns"] = r1.exec_time_ns

    # ---- host: global scalar reduction (the "all-reduce" of 3 scalars) ----
    C = np.zeros((128, 128), dtype=np.float64)
    Rall = np.empty((128, B), dtype=np.float64)
    for i in range(N_CORES):
        p = r1.results[i]["p1"].astype(np.float64)
        C += p[:, :128]
        Rall[:, i * B_LOC : (i + 1) * B_LOC] = p[:, 128:132]
    W0d = W0.astype(np.float64)
    M = W0d @ W0d.T
    S2 = float(np.vdot(M, C))
    T = Rall.T @ W0d  # [B, 512]
    S1 = float(T.sum())
    s = S1 / np.sqrt(max(S2, 1e-12))
    sjh2 = (s / J) * T
    n2 = float((sjh2 * sjh2).sum())
    sj2 = (sjh2 / np.sqrt(max(n2, 1e-12))).reshape(B, J, D)
    # As[b][din, j] = s * sum_dd W0[din, (j,dd)] * sj2[b, j, dd]
    A = np.einsum("dje,bje->bdj", W0d.reshape(DIN, J, D), sj2)
    As = (s * A).astype(bf16)  # [B, 128, 32]

    # ---- phase 2: logits/softmax/G ----
    nc2 = _get("p2")
    in2 = [
        {
            "ut": ut8s[i],
            "u2": u2s[i],
            "A": np.ascontiguousarray(
                As[i * B_LOC : (i + 1) * B_LOC].transpose(1, 0, 2).reshape(
                    DIN, B_LOC * J
                )
            ),
        }
        for i in range(N_CORES)
    ]
    r2 = run_bass_kernel_spmd(
        nc2, in2, core_ids=list(range(N_CORES)), trace=PROFILE
    )
    if PROFILE:
        LAST_TIMES["phase2_ns"] = r2.exec_time_ns

    # ---- host: fold G with W + squash ----
    W0r = W0d.reshape(DIN, J, D)
    out = np.empty((B, J, D), dtype=np.float32)
    for i in range(N_CORES):
        Gi = r2.results[i]["out"].astype(np.float64)  # [128, 128]
        for bl in range(B_LOC):
            Gb = Gi[32 * bl : 32 * bl + 32, :]  # [j, din]
            sjh3 = s * np.einsum("jd,djk->jk", Gb, W0r)
            s2 = (sjh3 * sjh3).sum(axis=-1, keepdims=True) + 1e-7
            out[i * B_LOC + bl] = (np.sqrt(s2) / (1.0 + s2)) * sjh3
    return out


# revision 16
# speedup vs baseline: 1.1885x; 1.0668x over previous
"""Trainium2 Bass kernel for nn_Capsule (dynamic routing capsule layer).

Math: with cij initialized to zeros, routing iteration 1 collapses to
cij = 1/32 (softmax of zeros), so the whole forward reduces to:
  T[b,j,d]   = sum_n u_hat[b,j,n,d]            (= rowsum(u[b]) @ W)
  S1         = sum(u_hat) = sum(T)
  S2         = sum(u_hat^2) = <W W^T, u^T u>   (feature Gram)
  s          = S1 * rsqrt(max(S2, 1e-12))      (global l2_normalize scalar)
  sjh2       = (s/32) * T ; sj2 = sjh2 * rsqrt(max(sum(sjh2^2), 1e-12))
  logits     = u @ As[b],  As[b][din,j] = s * sum_dd W[din,(j,dd)] sj2[b,j,dd]
  cij        = softmax_j(logits)
  G[b][j,:]  = sum_n cij[b,j,n] u[b,n,:]
  out        = squash(s * (G[b] fold W))
u_hat (256 MiB) is never materialized.  Sharding: data-parallel over
batch B (4 per core).  The cross-core reduction is 3 scalars worth of
partials (C [128x128] Gram + rowsums R), reduced on the host between
the two launches (in-kernel collectives cost ~65us here, far above the
two-launch overhead).  Layouts are host-swizzled so every DMA line is
>=2KB contiguous on both HBM and SBUF sides (the naive row-gather
yields 256B descriptor lines and ~65% of HBM bandwidth).  The logits
operand u^T is fp8 (softmax is near-uniform, |logit| <= 0.13, so fp8
quantization of u is harmless there); Gram and G operands stay bf16.
The fold-with-W + squash tail runs on the host (O(B*J*DIN*D) work) so
the second launch ends right after the G matmuls.
"""

import numpy as np

import concourse.bacc as bacc
import concourse.mybir as mybir
import concourse.tile as tile
from concourse.bass import ts
from concourse.bass_utils import run_bass_kernel_spmd

N_CORES = 8
B, N, DIN = 32, 4096, 128
J, D = 32, 16
K = J * D  # 512
B_LOC = B // N_CORES          # 4 batches per core
R_LOC = B_LOC * N             # 16384 rows per core
NCH = R_LOC // 128            # 128 chunks of 128 rows
CH_PER_B = N // 128           # 32 chunks per batch
NG1 = 8                       # phase-1 DMA groups (512 KiB each)
CHG1 = NCH // NG1             # 32 chunks per phase-1 group
NG = 8                        # phase-2 DMA groups
CHG = NCH // NG               # 16 chunks per phase-2 group
F32 = mybir.dt.float32
BF16 = mybir.dt.bfloat16
F8 = mybir.dt.float8e4
AX = mybir.AxisListType
ALU = mybir.AluOpType
ACTF = mybir.ActivationFunctionType

PROFILE = False
LAST_TIMES = {}

_CACHE = {}


def _new_bass():
    return bacc.Bacc(
        "TRN2",
        target_bir_lowering=False,
        debug=False,
        enable_asserts=True,
        num_devices=N_CORES,
    )


def _build_phase1():
    """Per core: C = sum_b u[b]^T u[b]  (feature Gram, [128,128]) and
    R[:, b] = sum_n u[b,n,:]  -> output [128, 132].

    Input u1 is host-swizzled [p, chunk, 129] bf16 where cols 0:128 are
    chunk rows and col 128 is a baked 1.0 (rides the Gram matmul to
    produce per-chunk rowsums in psum column 128)."""
    nc = _new_bass()
    u_d = nc.dram_tensor("u1", [128, NCH * 129], BF16, kind="ExternalInput")
    o_d = nc.dram_tensor("p1", [128, 132], F32, kind="ExternalOutput")

    with tile.TileContext(nc) as tc:
        with (
            tc.tile_pool(name="upool", bufs=1) as upool,
            tc.tile_pool(name="psp", bufs=1, space="PSUM") as psp,
            tc.tile_pool(name="sbp", bufs=1) as sbp,
        ):
            # HAM warm-up: ~4us of dummy matmuls while the u DMA is in
            # flight, so the PE clock is at 2.4GHz (not the cold 1.2GHz)
            # when the real matmuls start.
            wt = sbp.tile([128, 512], BF16, tag="wt", name="wt")
            nc.vector.memset(wt[:], 0.0)
            wp = psp.tile([64, 512], F32, tag="wp", name="wp")
            for _ in range(6):
                nc.tensor.matmul(wp[:], wt[:, 0:64], wt[:], start=True, stop=True)

            ugs = []
            for g in range(NG1):
                ug = upool.tile([128, CHG1 * 129], BF16, tag=f"ug{g}", name=f"ug{g}")
                ugs.append(ug)
                # alternate the two HWDGE rings; SWDGE (gpsimd) transfers
                # serialize at ~3us each and arrive far too late
                eng = nc.sync if g % 2 == 0 else nc.scalar
                eng.dma_start(ug[:], u_d.ap()[:, ts(g, CHG1 * 129)])

            cps = [
                psp.tile([128, 129], F32, tag=f"c{b}", name=f"c{b}")
                for b in range(B_LOC)
            ]

            for c in range(NCH):
                g, cl = divmod(c, CHG1)
                b = c // CH_PER_B
                view = ugs[g][:].rearrange("p (c e) -> p c e", e=129)[:, cl, :]
                nc.tensor.matmul(
                    cps[b][:],
                    view[:, 0:128],
                    view,
                    start=(c % CH_PER_B == 0),
                    stop=(c % CH_PER_B == CH_PER_B - 1),
                )

            outsb = sbp.tile([128, 132], F32, tag="outsb", name="outsb")
            nc.scalar.copy(outsb[:, 0:128], cps[0][:, 0:128])
            for b in range(1, B_LOC):
                nc.vector.tensor_add(
                    outsb[:, 0:128], outsb[:, 0:128], cps[b][:, 0:128]
                )
            for b in range(B_LOC):
                nc.scalar.copy(outsb[:, 128 + b : 129 + b], cps[b][:, 128:129])
            nc.sync.dma_start(o_d.ap(), outsb[:])

    nc.compile()
    return nc


def _build_phase2():
    """Per core: logits (fp8 u^T x bf16 As) -> softmax_j -> G -> out.

    out row layout: rows 32*bl+j hold G[b=core*4+bl][j, :] (din on the
    free axis).  Fold with W and squash happen on the host."""
    nc = _new_bass()
    ut_d = nc.dram_tensor("ut", [128, R_LOC], F8, kind="ExternalInput")
    u2_d = nc.dram_tensor("u2", [128, NCH * 128], BF16, kind="ExternalInput")
    a_d = nc.dram_tensor("A", [128, B_LOC * J], BF16, kind="ExternalInput")
    o_d = nc.dram_tensor("out", [128, 128], F32, kind="ExternalOutput")

    with tile.TileContext(nc) as tc:
        with (
            tc.tile_pool(name="const", bufs=1) as cstp,
            tc.tile_pool(name="utp", bufs=1) as utp,
            tc.tile_pool(name="u2p", bufs=1) as u2p,
            tc.tile_pool(name="expp", bufs=2) as expp,
            tc.tile_pool(name="cijp", bufs=3) as cijp,
            tc.tile_pool(name="zp", bufs=2) as zp,
            tc.tile_pool(name="sbt", bufs=1) as sbt,
            tc.tile_pool(name="plp", bufs=4, space="PSUM") as plp,
            tc.tile_pool(name="tlp", bufs=1, space="PSUM") as tlp,
        ):
            # HAM warm-up (see phase 1)
            wt = cstp.tile([128, 512], BF16, tag="wt", name="wt")
            nc.vector.memset(wt[:], 0.0)
            wp = tlp.tile([64, 512], F32, tag="wp", name="wp")
            for _ in range(6):
                nc.tensor.matmul(wp[:], wt[:, 0:64], wt[:], start=True, stop=True)

            # small load first so it doesn't queue behind the u loads
            a_sb = cstp.tile([128, B_LOC * J], BF16, tag="a_sb", name="a_sb")
            nc.scalar.dma_start(a_sb[:], a_d.ap())

            # graduated group sizes: the serial exp->softmax->G chain for the
            # LAST group runs after the final DMA byte, so the tail groups
            # shrink (16...16,8,4,4 chunks) to cut that exposed latency
            GROUPS = [(0, 16), (16, 16), (32, 16), (48, 16), (64, 16),
                      (80, 16), (96, 16), (112, 8), (120, 4), (124, 4)]
            NGV = len(GROUPS)

            # ut on the sync HWDGE ring; u2 on the scalar ring, but only the
            # first 3 issued up front — the rest are interleaved between the
            # EXPs below so a full ring never blocks an EXP behind it in the
            # scalar queue (SWDGE/gpsimd is not an option: its transfers
            # serialize at ~3us each and arrive far too late)
            uts, u2s = [], []
            for g, (c0, ng) in enumerate(GROUPS):
                ut = utp.tile([128, ng * 128], F8, tag=f"ut{g}", name=f"ut{g}")
                uts.append(ut)
                nc.sync.dma_start(ut[:], ut_d.ap()[:, c0 * 128 : (c0 + ng) * 128])
                u2 = u2p.tile([128, ng * 128], BF16, tag=f"u2{g}", name=f"u2{g}")
                u2s.append(u2)
            U2_HEAD = 2
            for g in range(U2_HEAD):
                c0, ng = GROUPS[g]
                nc.scalar.dma_start(u2s[g][:], u2_d.ap()[:, c0 * 128 : (c0 + ng) * 128])

            psg = tlp.tile([128, 128], F32, tag="psg", name="psg")  # G accum

            pls = [None] * NGV
            LAG = 2  # groups of logits emitted ahead of their softmax+G chain

            def emit_logits(g):
                c0, ng = GROUPS[g]
                pls[g] = plp.tile([128, 512], F32, tag="pl", name=f"pl{g}")
                for cl in range(ng):
                    b = (c0 + cl) // CH_PER_B
                    nc.tensor.matmul(
                        pls[g][:, ts(cl, J)],
                        uts[g][:, ts(cl, 128)],
                        a_sb[:, ts(b, J)],
                        start=True,
                        stop=True,
                    )

            def emit_chain(g):
                # softmax over j (free axis) + G matmuls for group g
                c0, ng = GROUPS[g]
                eg = expp.tile([128, 512], BF16, tag="eg", name=f"eg{g}")
                nc.scalar.activation(eg[:, : ng * J], pls[g][:, : ng * J], ACTF.Exp)
                zg = zp.tile([128, 16], BF16, tag="zg", name=f"zg{g}")
                zr = zp.tile([128, 16], BF16, tag="zr", name=f"zr{g}")
                cg = cijp.tile([128, 512], BF16, tag="cg", name=f"cg{g}")
                with nc.allow_low_precision(
                    reason="softmax denominator: 32-way sum of O(1) exps; "
                    "bf16 keeps the full DVE 2x 16-bit rate"
                ):
                    nc.vector.reduce_sum(
                        zg[:, :ng],
                        eg[:, : ng * J].rearrange("p (c j) -> p c j", j=J),
                        axis=AX.X,
                    )
                    nc.vector.reciprocal(zr[:, :ng], zg[:, :ng])
                    # the big elementwise multiply runs on the otherwise-idle
                    # gpsimd engine so the vector engine stays off the
                    # critical path
                    nc.gpsimd.tensor_tensor(
                        cg[:, : ng * J].rearrange("p (c j) -> p c j", j=J),
                        eg[:, : ng * J].rearrange("p (c j) -> p c j", j=J),
                        zr[:, :ng].unsqueeze(2).broadcast_to([128, ng, J]),
                        op=ALU.mult,
                    )
                for cc in range(ng):
                    c2 = c0 + cc
                    b2 = c2 // CH_PER_B
                    nc.tensor.matmul(
                        psg[ts(b2, J), :],
                        cg[:, ts(cc, J)],
                        u2s[g][:, ts(cc, 128)],
                        start=(c2 % CH_PER_B == 0),
                        stop=(c2 % CH_PER_B == CH_PER_B - 1),
                        tile_position=(0, 32 * b2),
                    )

            def emit_u2_dma(g):
                if U2_HEAD <= g < NGV:
                    c0, ng = GROUPS[g]
                    nc.scalar.dma_start(
                        u2s[g][:], u2_d.ap()[:, c0 * 128 : (c0 + ng) * 128]
                    )

            for g in range(NGV):
                emit_logits(g)
                if g >= LAG:
                    emit_chain(g - LAG)
                    emit_u2_dma(g - LAG + U2_HEAD)
            for g in range(NGV - LAG, NGV):
                emit_chain(g)
                emit_u2_dma(g + U2_HEAD)

            gout = sbt.tile([128, 128], F32, tag="gout", name="gout")
            nc.scalar.copy(gout[:], psg[:])
            nc.sync.dma_start(o_d.ap(), gout[:])

    nc.compile()
    return nc


def _get(name):
    if name not in _CACHE:
        if name == "p1":
            _CACHE[name] = _build_phase1()
        else:
            _CACHE[name] = _build_phase2()
    return _CACHE[name]


def kernel(u, W):
    import ml_dtypes

    bf16 = ml_dtypes.bfloat16
    f8 = ml_dtypes.float8_e4m3
    u = np.ascontiguousarray(u, dtype=np.float32)
    W = np.ascontiguousarray(W, dtype=np.float32)
    W0 = np.ascontiguousarray(W[0])  # [128, 512]

    u1s, u2s, ut8s = [], [], []
    for i in range(N_CORES):
        shf = u[i * B_LOC : (i + 1) * B_LOC].reshape(R_LOC, DIN)
        shc = shf.astype(bf16).reshape(NCH, 128, DIN)  # [c, p, d]
        sw = shc.transpose(1, 0, 2)  # [p, c, d]
        u1 = np.empty((128, NCH, 129), dtype=bf16)
        u1[:, :, 0:128] = sw
        u1[:, :, 128] = bf16(1.0)
        u1s.append(np.ascontiguousarray(u1.reshape(128, NCH * 129)))
        u2s.append(np.ascontiguousarray(sw.reshape(128, NCH * 128)))
        ut8s.append(np.ascontiguousarray(shf.T).astype(f8))

    # ---- phase 1: per-core Gram + rowsums ----
    nc1 = _get("p1")
    r1 = run_bass_kernel_spmd(
        nc1,
        [{"u1": u1s[i]} for i in range(N_CORES)],
        core_ids=list(range(N_CORES)),
        trace=PROFILE,
    )
    if PROFILE:
        LAST_TIMES["phase1_ns"] = r1.exec_time_ns

    # ---- host: global scalar reduction (the "all-reduce" of 3 scalars) ----
    C = np.zeros((128, 128), dtype=np.float64)
    Rall = np.empty((128, B), dtype=np.float64)
    for i in range(N_CORES):
        p = r1.results[i]["p1"].astype(np.float64)
        C += p[:, :128]
        Rall[:, i * B_LOC : (i + 1) * B_LOC] = p[:, 128:132]
    W0d = W0.astype(np.float64)
    M = W0d @ W0d.T
    S2 = float(np.vdot(M, C))
    T = Rall.T @ W0d  # [B, 512]
    S1 = float(T.sum())
    s = S1 / np.sqrt(max(S2, 1e-12))
    sjh2 = (s / J) * T
    n2 = float((sjh2 * sjh2).sum())
    sj2 = (sjh2 / np.sqrt(max(n2, 1e-12))).reshape(B, J, D)
    # As[b][din, j] = s * sum_dd W0[din, (j,dd)] * sj2[b, j, dd]
    A = np.einsum("dje,bje->bdj", W0d.reshape(DIN, J, D), sj2)
    As = (s * A).astype(bf16)  # [B, 128, 32]

    # ---- phase 2: logits/softmax/G ----
    nc2 = _get("p2")
    in2 = [
        {
            "ut": ut8s[i],
            "u2": u2s[i],
            "A": np.ascontiguousarray(
                As[i * B_LOC : (i + 1) * B_LOC].transpose(1, 0, 2).reshape(
                    DIN, B_LOC * J
                )
            ),
        }
        for i in range(N_CORES)
    ]
    r2 = run_bass_kernel_spmd(
        nc2, in2, core_ids=list(range(N_CORES)), trace=PROFILE
    )
    if PROFILE:
        LAST_TIMES["phase2_ns"] = r2.exec_time_ns

    # ---- host: fold G with W + squash ----
    W0r = W0d.reshape(DIN, J, D)
    out = np.empty((B, J, D), dtype=np.float32)
    for i in range(N_CORES):
        Gi = r2.results[i]["out"].astype(np.float64)  # [128, 128]
        for bl in range(B_LOC):
            Gb = Gi[32 * bl : 32 * bl + 32, :]  # [j, din]
            sjh3 = s * np.einsum("jd,djk->jk", Gb, W0r)
            s2 = (sjh3 * sjh3).sum(axis=-1, keepdims=True) + 1e-7
            out[i * B_LOC + bl] = (np.sqrt(s2) / (1.0 + s2)) * sjh3
    return out
